# revision 11
# baseline (speedup 1.0000x reference)
"""BiasedAxialAttention (row-attention path) distributed over 8 TRN2 NeuronCores.

Sharding: outer (non-attended) L axis "n" (= p axis 1 after the reference's
permute, = pair axis 2) split into 8 slices of 48 rows.

Per-core dataflow (all shapes hardcoded for B=1, L=384, D=128, H=4, DH=32):
  phase 1: LN(x_qkv) -> q,k projections emitted directly in the shuffled
           [(k*4+s), i] layout via scatter-column weights (4 accumulating
           matmuls per 4-n group), v projection in [j, (h,d)] layout,
           b = bias @ Wb^T via PE-transposed bias tiles,
           logits[i,j,h] accumulated over 12 groups at K=128.
  phase 2: ReduceScatter(logits over i) -> +b -> softmax(j) -> AllGather.
           gate path (LN + Wg + sigmoid) overlaps the collectives.
  phase 3: transpose attn -> AV (per-head 32-row strips of one PSUM tile),
           gate multiply (+folded cv via softmax-sums-to-1), Wo, +bo, DMA out.
"""

import math

import numpy as np
import ml_dtypes

H, DH, D, L = 4, 32, 128, 384
NCORES = 8
R = L // NCORES  # 48
SCALING = 1.0 / math.sqrt(DH)
KSCALE = 1.0 / math.sqrt(L)
EPS = 1e-5
NG = R // 4  # 12 groups of 4 n-rows

_CACHE = {}


def _build_graph():
    import concourse.bass as bass
    import concourse.tile as tile
    from concourse import bacc, mybir

    f32 = mybir.dt.float32
    bf16 = mybir.dt.bfloat16
    Exp = mybir.ActivationFunctionType.Exp
    Identity = mybir.ActivationFunctionType.Identity
    Sigmoid = mybir.ActivationFunctionType.Sigmoid
    Sqrt = mybir.ActivationFunctionType.Sqrt
    sub = mybir.AluOpType.subtract
    mult = mybir.AluOpType.mult
    add = mybir.AluOpType.add

    nc = bacc.Bacc(
        "TRN2", target_bir_lowering=False, debug=False, num_devices=NCORES
    )

    # ---- external params (per-core shards + shared prepped weights) ----
    x_qkv = nc.declare_dram_parameter("x_qkv", [R, L, D], f32, isOutput=False)
    x_gate = nc.declare_dram_parameter("x_gate", [R, L, D], f32, isOutput=False)
    bias_c = nc.declare_dram_parameter("bias_c", [R, L, D], f32, isOutput=False)
    wq_scat = nc.declare_dram_parameter("wq_scat", [16, D, D], bf16, isOutput=False)
    wk_scat = nc.declare_dram_parameter("wk_scat", [16, D, D], bf16, isOutput=False)
    wv_t = nc.declare_dram_parameter("wv_t", [D, D], bf16, isOutput=False)
    wg_t = nc.declare_dram_parameter("wg_t", [D, D], bf16, isOutput=False)
    wo_t = nc.declare_dram_parameter("wo_t", [D, D], bf16, isOutput=False)
    wb_t = nc.declare_dram_parameter("wb_t", [D, H], f32, isOutput=False)
    cq_sh = nc.declare_dram_parameter("cq_sh", [H, D], f32, isOutput=False)
    ck_sh = nc.declare_dram_parameter("ck_sh", [H, D], f32, isOutput=False)
    cg_v = nc.declare_dram_parameter("cg_v", [D], f32, isOutput=False)
    cv_v = nc.declare_dram_parameter("cv_v", [D], f32, isOutput=False)
    bo_v = nc.declare_dram_parameter("bo_v", [D], f32, isOutput=False)
    out_p = nc.declare_dram_parameter("out", [R, L, D], f32, isOutput=True)

    # ---- internal DRAM (collective bounces; outs must be Shared) ----
    logits_dram = nc.dram_tensor("logits_dram", [L, H, L], f32)
    rs_out = nc.dram_tensor("rs_out", [R, H, L], f32)
    attn_bounce = nc.dram_tensor("attn_bounce", [R, H, L], f32)
    attn_full = nc.dram_tensor("attn_full", [L, H, L], f32, addr_space="Shared")
    b_dram = nc.dram_tensor("b_dram", [H, R * L], f32)
    groups = [list(range(NCORES))]

    with tile.TileContext(nc) as tc:
        from contextlib import ExitStack

        with ExitStack() as top:
            consts = top.enter_context(tc.tile_pool(name="consts", bufs=1))

            # constant tiles
            id_bf = consts.tile([D, D], bf16)
            id_f32 = consts.tile([D, D], f32)
            wqs_sb = consts.tile([D, 16, D], bf16)   # [d, (h,s), P]
            wks_sb = consts.tile([D, 16, D], bf16)
            wv_sb = consts.tile([D, D], bf16)
            wg_sb = consts.tile([D, D], bf16)
            wo_sb = consts.tile([D, D], bf16)
            wb_sb = consts.tile([D, H], f32)
            cq_sb = consts.tile([D, H], f32)         # per-partition bias, col h
            ck_sb = consts.tile([D, H], f32)
            cg_sb = consts.tile([D, 1], f32)
            cv_sb = consts.tile([D, 1], f32)
            bo_bc = consts.tile([D, D], f32)         # bo broadcast along partitions
            eps_sb = consts.tile([D, 1], f32)

            from concourse.masks import make_identity

            make_identity(nc, id_bf)
            make_identity(nc, id_f32)
            nc.sync.dma_start(out=wqs_sb, in_=wq_scat.ap().rearrange("s d p -> d s p"))
            nc.sync.dma_start(out=wks_sb, in_=wk_scat.ap().rearrange("s d p -> d s p"))
            nc.sync.dma_start(out=wv_sb, in_=wv_t[:, :])
            nc.sync.dma_start(out=wg_sb, in_=wg_t[:, :])
            nc.sync.dma_start(out=wo_sb, in_=wo_t[:, :])
            nc.sync.dma_start(out=wb_sb, in_=wb_t[:, :])
            nc.sync.dma_start(out=cq_sb, in_=cq_sh.ap().rearrange("h d -> d h"))
            nc.sync.dma_start(out=ck_sb, in_=ck_sh.ap().rearrange("h d -> d h"))
            nc.sync.dma_start(out=cg_sb, in_=cg_v.ap().unsqueeze(1))
            nc.sync.dma_start(out=cv_sb, in_=cv_v.ap().unsqueeze(1))
            nc.sync.dma_start(
                out=bo_bc, in_=bo_v.ap().unsqueeze(0).broadcast_to((D, D))
            )
            nc.vector.memset(eps_sb, EPS)

            # persistent stores
            stores = top.enter_context(tc.tile_pool(name="stores", bufs=1))
            v_st = stores.tile([D, 3, R, D], bf16)      # [j, jc, y, (h,d)]
            g_st = stores.tile([D, R, L], bf16)         # [(h,d), y, x]
            b_sb = stores.tile([D, H, 3 * R], f32)      # [rowchunk-part, h, tile]

            # ---------------- phase 1: QKV + b + logits ----------------
            qk_ctx = ExitStack()
            qk_st = qk_ctx.enter_context(tc.tile_pool(name="qk_st", bufs=1))
            qsh = qk_st.tile([D, H, NG, L], bf16)   # [(k,s), h, g, i]
            ksh = qk_st.tile([D, H, NG, L], bf16)
            with ExitStack() as ph1:
                xin_p = ph1.enter_context(tc.tile_pool(name="xin", bufs=3))
                st_p = ph1.enter_context(tc.tile_pool(name="stats", bufs=4))
                xh_p = ph1.enter_context(tc.tile_pool(name="xh", bufs=3))
                xt_p = ph1.enter_context(
                    tc.tile_pool(name="xt", bufs=1, space="PSUM")
                )
                xts_p = ph1.enter_context(tc.tile_pool(name="xts", bufs=5))
                vps_p = ph1.enter_context(
                    tc.tile_pool(name="vps", bufs=1, space="PSUM")
                )
                slab_p = ph1.enter_context(
                    tc.tile_pool(name="slab", bufs=4, space="PSUM")
                )
                bt_p = ph1.enter_context(tc.tile_pool(name="bt", bufs=2))
                btp_p = ph1.enter_context(
                    tc.tile_pool(name="btp", bufs=1, space="PSUM")
                )
                bps_p = ph1.enter_context(
                    tc.tile_pool(name="bps", bufs=1, space="PSUM")
                )

                def ln_to_xhatT(src_dram, n, engine_idx):
                    """LN rows of src[n] and return [d, 384] bf16 transposed tile."""
                    xin = xin_p.tile([D, 3, D], f32, tag="xin")
                    nc.sync.dma_start(
                        out=xin,
                        in_=src_dram[n].rearrange("(t p) d -> p t d", p=D),
                    )
                    stt = st_p.tile([D, 3, 6], f32, tag="st")
                    mv = st_p.tile([D, 3, 2], f32, tag="mv")
                    for t in range(3):
                        nc.vector.bn_stats(out=stt[:, t, :], in_=xin[:, t, :])
                        nc.vector.bn_aggr(out=mv[:, t, :], in_=stt[:, t, :])
                    sd = st_p.tile([D, 3], f32, tag="sd")
                    nc.scalar.activation(
                        out=sd, in_=mv[:, :, 1], func=Sqrt, bias=eps_sb, scale=1.0
                    )
                    istd = st_p.tile([D, 3], f32, tag="istd")
                    nc.vector.reciprocal(out=istd, in_=sd)
                    xh = xh_p.tile([D, 3, D], bf16, tag="xh")
                    for t in range(3):
                        nc.gpsimd.tensor_scalar(
                            out=xh[:, t, :],
                            in0=xin[:, t, :],
                            scalar1=mv[:, t, 0:1],
                            scalar2=istd[:, t : t + 1],
                            op0=sub,
                            op1=mult,
                        )
                    xt = xt_p.tile([D, L], bf16, tag="xt")
                    for t in range(3):
                        nc.tensor.transpose(
                            out=xt[:, t * D : (t + 1) * D], in_=xh[:, t, :],
                            identity=id_bf,
                        )
                    xts = xts_p.tile([D, L], bf16, tag="xts")
                    if engine_idx % 2 == 0:
                        nc.vector.tensor_copy(out=xts, in_=xt)
                    else:
                        nc.scalar.copy(out=xts, in_=xt)
                    return xts

                for g in range(NG):
                    xts_g = []
                    psq = [slab_p.tile([D, L], f32, tag="slab", name=f"psq_{g}_{h}") for h in range(H)]
                    for s in range(4):
                        n = 4 * g + s
                        xts = ln_to_xhatT(x_qkv, n, n)
                        xts_g.append(xts)
                        # v projection: [j-chunk, (h,d)] per chunk
                        for jc in range(3):
                            vps = vps_p.tile([D, D], f32, tag="vps")
                            nc.tensor.matmul(
                                vps,
                                xts[:, jc * D : (jc + 1) * D],
                                wv_sb,
                                start=True,
                                stop=True,
                            )
                            nc.vector.tensor_copy(
                                out=v_st[:, jc, n, :], in_=vps
                            )
                        # q scattered projections accumulate into 4 head slabs
                        for h in range(H):
                            nc.tensor.matmul(
                                psq[h],
                                wqs_sb[:, h * 4 + s, :],
                                xts,
                                start=(s == 0),
                                stop=(s == 3),
                            )
                    for h in range(H):
                        nc.scalar.activation(
                            out=qsh[:, h, g, :], in_=psq[h], func=Identity,
                            bias=cq_sb[:, h : h + 1], scale=1.0,
                        )
                    psk = [slab_p.tile([D, L], f32, tag="slab", name=f"psk_{g}_{h}") for h in range(H)]
                    for s in range(4):
                        for h in range(H):
                            nc.tensor.matmul(
                                psk[h],
                                wks_sb[:, h * 4 + s, :],
                                xts_g[s],
                                start=(s == 0),
                                stop=(s == 3),
                            )
                    for h in range(H):
                        nc.scalar.activation(
                            out=ksh[:, h, g, :], in_=psk[h], func=Identity,
                            bias=ck_sb[:, h : h + 1], scale=1.0,
                        )

                    # bias path: 12 row-tiles per group (rows (i,j), i-major)
                    bpp = bps_p.tile([D, 12, H], f32, tag="bps")
                    for u in range(12):
                        a = 12 * g + u
                        bin_t = bt_p.tile([D, D], f32, tag="bin")
                        nc.sync.dma_start(
                            out=bin_t,
                            in_=bias_c.ap()
                            .rearrange("r l d -> (r l) d")
                            .rearrange("(a p) d -> a p d", p=D)[a],
                        )
                        btp = btp_p.tile([D, D], f32, tag="btp")
                        nc.tensor.transpose(out=btp, in_=bin_t, identity=id_f32)
                        bts = bt_p.tile([D, D], f32, tag="bts")
                        if u % 2 == 0:
                            nc.vector.tensor_copy(out=bts, in_=btp)
                        else:
                            nc.scalar.copy(out=bts, in_=btp)
                        nc.tensor.matmul(
                            bpp[:, u, :], bts, wb_sb, start=True, stop=True
                        )
                    nc.vector.tensor_copy(
                        out=b_sb[:, :, 12 * g : 12 * (g + 1)],
                        in_=bpp.rearrange("p u h -> p h u"),
                    )


            # logits: [i-chunk, j] per head, K=128 over 12 groups
            with ExitStack() as phl:
                lg_p = phl.enter_context(
                    tc.tile_pool(name="lgp", bufs=4, space="PSUM")
                )
                ls_p = phl.enter_context(tc.tile_pool(name="lsb", bufs=2))
                for ic in range(3):
                    lsb = ls_p.tile([D, H, L], f32, tag="lsb")
                    for h in range(H):
                        pl = lg_p.tile([D, L], f32, tag="lg")
                        for g in range(NG):
                            nc.tensor.matmul(
                                pl,
                                qsh[:, h, g, ic * D : (ic + 1) * D],
                                ksh[:, h, g, :],
                                start=(g == 0),
                                stop=(g == NG - 1),
                            )
                        if h % 2 == 0:
                            nc.vector.tensor_copy(out=lsb[:, h, :], in_=pl)
                        else:
                            nc.scalar.copy(out=lsb[:, h, :], in_=pl)
                    nc.sync.dma_start(
                        out=logits_dram[ic * D : (ic + 1) * D, :, :], in_=lsb
                    )
            qk_ctx.close()

            # b_sb -> b_dram [h, rows] (partition pairs with 4-byte dst dim)
            b_dram_w = bass.AP(
                tensor=b_dram.ap().tensor,
                offset=0,
                ap=[[1, D], [R * L, H], [D, 3 * R]],
            )
            nc.sync.dma_start(out=b_dram_w, in_=b_sb)

            # ---------------- collective 1: ReduceScatter over i ----------------
            nc.gpsimd.collective_compute(
                "ReduceScatter",
                add,
                replica_groups=groups,
                ins=[logits_dram.ap().opt()],
                outs=[rs_out.ap().opt()],
            )

            # ---------------- gate path (overlaps collectives) ----------------
            with ExitStack() as ph2:
                xin_p = ph2.enter_context(tc.tile_pool(name="xin2", bufs=3))
                st_p = ph2.enter_context(tc.tile_pool(name="stats2", bufs=4))
                xh_p = ph2.enter_context(tc.tile_pool(name="xh2", bufs=3))
                xt_p = ph2.enter_context(
                    tc.tile_pool(name="xt2", bufs=2, space="PSUM")
                )
                xts_p = ph2.enter_context(tc.tile_pool(name="xts2", bufs=2))
                gp_p = ph2.enter_context(
                    tc.tile_pool(name="gp", bufs=2, space="PSUM")
                )
                for y in range(R):
                    xin = xin_p.tile([D, 3, D], f32, tag="xin")
                    nc.sync.dma_start(
                        out=xin,
                        in_=x_gate[y].rearrange("(t p) d -> p t d", p=D),
                    )
                    stt = st_p.tile([D, 3, 6], f32, tag="st")
                    mv = st_p.tile([D, 3, 2], f32, tag="mv")
                    for t in range(3):
                        nc.vector.bn_stats(out=stt[:, t, :], in_=xin[:, t, :])
                        nc.vector.bn_aggr(out=mv[:, t, :], in_=stt[:, t, :])
                    sd = st_p.tile([D, 3], f32, tag="sd")
                    nc.scalar.activation(
                        out=sd, in_=mv[:, :, 1], func=Sqrt, bias=eps_sb, scale=1.0
                    )
                    istd = st_p.tile([D, 3], f32, tag="istd")
                    nc.vector.reciprocal(out=istd, in_=sd)
                    xh = xh_p.tile([D, 3, D], bf16, tag="xh")
                    for t in range(3):
                        nc.gpsimd.tensor_scalar(
                            out=xh[:, t, :],
                            in0=xin[:, t, :],
                            scalar1=mv[:, t, 0:1],
                            scalar2=istd[:, t : t + 1],
                            op0=sub,
                            op1=mult,
                        )
                    xt = xt_p.tile([D, L], bf16, tag="xt")
                    for t in range(3):
                        nc.tensor.transpose(
                            out=xt[:, t * D : (t + 1) * D], in_=xh[:, t, :],
                            identity=id_bf,
                        )
                    xts = xts_p.tile([D, L], bf16, tag="xts")
                    nc.vector.tensor_copy(out=xts, in_=xt)
                    gp = gp_p.tile([D, L], f32, tag="gp")
                    nc.tensor.matmul(gp, wg_sb, xts, start=True, stop=True)
                    nc.scalar.activation(
                        out=g_st[:, y, :], in_=gp, func=Sigmoid, bias=cg_sb,
                        scale=1.0,
                    )

            # ---------------- softmax on the i-shard ----------------
            with ExitStack() as ph3:
                sm_p = ph3.enter_context(tc.tile_pool(name="sm", bufs=1))
                rs_sb = sm_p.tile([R, H, L], f32)
                b2_sb = sm_p.tile([R, H, L], f32)
                nc.sync.dma_start(out=rs_sb, in_=rs_out[:, :, :])
                nc.sync.dma_start(
                    out=b2_sb,
                    in_=b_dram.ap().rearrange("h (i j) -> i h j", i=R),
                )
                ex_in = sm_p.tile([R, H, L], f32)
                nc.vector.tensor_add(out=ex_in, in0=rs_sb, in1=b2_sb)
                exp_sb = sm_p.tile([R, H, L], f32)
                sums = sm_p.tile([R, H], f32)
                for h in range(H):
                    nc.scalar.activation(
                        out=exp_sb[:, h, :], in_=ex_in[:, h, :], func=Exp,
                        accum_out=sums[:, h : h + 1],
                    )
                rsum = sm_p.tile([R, H], f32)
                nc.vector.reciprocal(out=rsum, in_=sums)
                attn_sb = sm_p.tile([R, H, L], f32)
                for h in range(H):
                    nc.gpsimd.tensor_scalar(
                        out=attn_sb[:, h, :],
                        in0=exp_sb[:, h, :],
                        scalar1=rsum[:, h : h + 1],
                        scalar2=None,
                        op0=mult,
                    )
                nc.sync.dma_start(out=attn_bounce[:, :, :], in_=attn_sb)

            # ---------------- collective 2: AllGather attn ----------------
            nc.gpsimd.collective_compute(
                "AllGather",
                mybir.AluOpType.bypass,
                replica_groups=groups,
                ins=[attn_bounce.ap().opt()],
                outs=[attn_full.ap().opt()],
            )

            # ---------------- phase 3: attn^T, AV, gate, Wo ----------------
            tc.strict_bb_all_engine_barrier()
            with ExitStack() as ph4:
                at_in_p = ph4.enter_context(tc.tile_pool(name="atin", bufs=3))
                at_ps_p = ph4.enter_context(
                    tc.tile_pool(name="atps", bufs=2, space="PSUM")
                )
                at_st = ph4.enter_context(tc.tile_pool(name="atst", bufs=1))
                attnT = at_st.tile([D, H, 3, L], bf16)  # [j, h, jc, x]
                at_in = [at_in_p.tile([D, H, L], f32, tag="atin", name=f"at_in_{i}") for i in range(3)]
                for ic in range(3):
                    nc.sync.dma_start(
                        out=at_in[ic], in_=attn_full[ic * D : (ic + 1) * D, :, :]
                    )
                for h in range(H):
                    for jc in range(3):
                        pt = at_ps_p.tile([D, L], f32, tag="atps")
                        for ic in range(3):
                            nc.tensor.transpose(
                                out=pt[:, ic * D : (ic + 1) * D],
                                in_=at_in[ic][:, h, jc * D : (jc + 1) * D],
                                identity=id_f32,
                            )
                        if jc % 2 == 0:
                            nc.vector.tensor_copy(out=attnT[:, h, jc, :], in_=pt)
                        else:
                            nc.scalar.copy(out=attnT[:, h, jc, :], in_=pt)

                av_p = ph4.enter_context(
                    tc.tile_pool(name="av", bufs=3, space="PSUM")
                )
                gt_p = ph4.enter_context(tc.tile_pool(name="gt", bufs=3))
                wo_ps = ph4.enter_context(
                    tc.tile_pool(name="wops", bufs=3, space="PSUM")
                )
                os_p = ph4.enter_context(tc.tile_pool(name="osb", bufs=3))
                for y in range(R):
                    pav = av_p.tile([D, L], f32, tag="av")
                    for h in range(H):
                        for jc in range(3):
                            nc.tensor.matmul(
                                pav[h * DH : (h + 1) * DH, :],
                                v_st[:, jc, y, h * DH : (h + 1) * DH],
                                attnT[:, h, jc, :],
                                start=(jc == 0),
                                stop=(jc == 2),
                                tile_position=(0, h * DH),
                            )
                    gated = gt_p.tile([D, L], bf16, tag="gt")
                    nc.vector.scalar_tensor_tensor(
                        out=gated,
                        in0=pav,
                        scalar=cv_sb,
                        in1=g_st[:, y, :],
                        op0=add,
                        op1=mult,
                    )
                    osb = os_p.tile([D, 3, D], f32, tag="osb")
                    for xc in range(3):
                        pwo = wo_ps.tile([D, D], f32, tag="wops")
                        nc.tensor.matmul(
                            pwo,
                            gated[:, xc * D : (xc + 1) * D],
                            wo_sb,
                            start=True,
                            stop=True,
                        )
                        nc.vector.tensor_add(
                            out=osb[:, xc, :], in0=pwo, in1=bo_bc
                        )
                    nc.sync.dma_start(
                        out=out_p[y].rearrange("(xc p) d -> p xc d", p=D),
                        in_=osb,
                    )

    nc.compile()
    return nc


def _prep_host(inputs):
    """Host-side: shard inputs, fold LN scale/bias + constants into weights."""
    f32 = np.float32
    bf = ml_dtypes.bfloat16
    pair = np.ascontiguousarray(np.asarray(inputs["pair"], f32)[0])
    bias = np.ascontiguousarray(np.asarray(inputs["bias"], f32)[0])
    ln_scale = np.asarray(inputs["ln_scale"], f32)
    ln_bias = np.asarray(inputs["ln_bias"], f32)
    Wq = np.asarray(inputs["Wq"], f32)
    Wk = np.asarray(inputs["Wk"], f32)
    Wv = np.asarray(inputs["Wv"], f32)
    Wb = np.asarray(inputs["Wb"], f32)
    Wg = np.asarray(inputs["Wg"], f32)
    bg = np.asarray(inputs["bg"], f32)
    Wo = np.asarray(inputs["Wo"], f32)
    bo = np.asarray(inputs["bo"], f32)

    Wq_eff = Wq * ln_scale[None, :] * SCALING
    Wk_eff = Wk * ln_scale[None, :] * KSCALE
    cq = (Wq @ ln_bias) * SCALING
    ck = (Wk @ ln_bias) * KSCALE

    def scat(W_eff):
        w = np.zeros((16, D, D), f32)
        for h in range(H):
            for s in range(4):
                for kk in range(DH):
                    w[h * 4 + s, :, kk * 4 + s] = W_eff[h * DH + kk, :]
        return w.astype(bf)

    wq_scat = scat(Wq_eff)
    wk_scat = scat(Wk_eff)
    cq_sh = np.zeros((H, D), f32)
    ck_sh = np.zeros((H, D), f32)
    for h in range(H):
        for s in range(4):
            for kk in range(DH):
                cq_sh[h, kk * 4 + s] = cq[h * DH + kk]
                ck_sh[h, kk * 4 + s] = ck[h * DH + kk]

    shared = {
        "wq_scat": wq_scat,
        "wk_scat": wk_scat,
        "wv_t": (Wv * ln_scale[None, :]).T.astype(bf).copy(),
        "wg_t": (Wg * ln_scale[None, :]).T.astype(bf).copy(),
        "wo_t": Wo.T.astype(bf).copy(),
        "wb_t": Wb.T.astype(f32).copy(),
        "cq_sh": cq_sh,
        "ck_sh": ck_sh,
        "cg_v": (Wg @ ln_bias + bg).astype(f32),
        "cv_v": (Wv @ ln_bias).astype(f32),
        "bo_v": bo.astype(f32),
    }
    in_maps = []
    for c in range(NCORES):
        sl = slice(c * R, (c + 1) * R)
        m = dict(shared)
        m["x_qkv"] = np.ascontiguousarray(pair[:, sl, :].transpose(1, 0, 2))
        m["x_gate"] = np.ascontiguousarray(pair[sl, :, :])
        m["bias_c"] = np.ascontiguousarray(bias[sl, :, :])
        in_maps.append(m)
    return in_maps


def kernel(**inputs):
    import os
    from concourse.bass_utils import run_bass_kernel_spmd

    in_maps = _prep_host(inputs)
    if "nc" not in _CACHE:
        _CACHE["nc"] = _build_graph()
    nc = _CACHE["nc"]
    kw = {}
    if os.environ.get("BAX_TRACE"):
        kw = dict(trace=True, tmpdir=os.environ.get("BAX_TRACE_DIR") or None)
    res = run_bass_kernel_spmd(nc, in_maps, list(range(NCORES)), **kw)
    _CACHE["last_result"] = res
    out = np.zeros((1, L, L, D), np.float32)
    for c in range(NCORES):
        out[0, c * R : (c + 1) * R, :, :] = res.results[c]["out"]
    return out


if __name__ == "__main__":
    nc = _build_graph()
    print("graph built ok")
